# revision 9
# baseline (speedup 1.0000x reference)
"""Trainium2 Bass kernel for a 3-hop GAT + global update, SPMD over 8 NeuronCores.

Math: per hop, q = x @ Wq[h] + bq[h]; attention logit for edge e is
logit_e = q[s_e]@wa1 + q[r_e]@wa2 + ba. Within a receiver's softmax segment the
receiver term and ba are constant, so they cancel:
    w_e = exp(a1[s_e]) / sum_{e' in r} exp(a1[s_e']),  a1 = q@wa1 + bq@wa1
    x_new[r] = (sum_e v[s_e] * q[s_e]) / (sum_e v[s_e]),  v = exp(a1)
So each hop reduces to: per-node [q*v | v] rows (129 floats), then a segmented
sum of gathered sender rows per receiver, then divide + leaky_relu.

Distribution: receivers are sharded across 8 cores. Nodes are globally sorted
by in-degree and tiles of 128 are snake-dealt to cores (load balance + uniform
per-tile ELL slot counts across cores, required for SPMD). Each hop:
  Phase A: each core computes q/v/y rows for its own 6272 nodes
  Phase B: AllGather replicates the y table (50176 x 129 f32)
  Phase C: per receiver tile, indirect-DMA gather sender rows slot by slot
           (ELL format, host-built index table) and accumulate into PSUM via
           identity matmuls; divide by the summed v (+1e-30) and leaky_relu.
Final: per-graph aggregation via one-hot matmuls, AllReduce, tiny dense layer.
"""
import numpy as np

import concourse.bass as bass
import concourse.mybir as mybir
from concourse import tile
from concourse.bass_utils import run_bass_kernel_spmd
from concourse.masks import make_identity

P = 128
NC = 8
F32 = mybir.dt.float32
I32 = mybir.dt.int32
AF = mybir.ActivationFunctionType
EPS = 1e-30
LRELU_ALPHA = 0.01


# ---------------------------------------------------------------------------
# wait legalization: walrus rejects instructions with more semaphore waits
# than their ISA struct has slots. Hoist excess waits onto same-engine NoOps.
def _legalize_waits(nc, max_default=1, max_matmul=1):
    inserted = 0
    for bb in nc.main_func.blocks:
        out = []
        changed = False
        for inst in bb.instructions:
            si = getattr(inst, "sync_info", None)
            waits = list(si.on_wait) if si is not None and si.on_wait else []
            limit = max_matmul if isinstance(inst, mybir.InstMatmult) else max_default
            if len(waits) > limit:
                excess = waits[: len(waits) - limit]
                keep = waits[len(waits) - limit:]
                for i in range(0, len(excess), max_default):
                    chunk = excess[i: i + max_default]
                    nop = mybir.InstNoOp(
                        name=nc.get_next_instruction_name(),
                        sync_info=mybir.SyncInfo(on_wait=chunk, on_update=[]),
                        bass_nofuse=True,
                        engine=inst.engine,
                    )
                    out.append(nop)
                    inserted += 1
                si.on_wait = keep
                inst.sync_info = si
                changed = True
            out.append(inst)
        if changed:
            bb.instructions = out
    return inserted


# ---------------------------------------------------------------------------
# host-side sharding / ELL construction
def _plan(senders, receivers, node_graph_idx, N, tiles_per_core):
    TPC = tiles_per_core
    NT = NC * TPC
    NPAD = NT * P
    SHARD = TPC * P
    E = senders.shape[0]

    deg = np.bincount(receivers, minlength=N)
    order = np.argsort(-deg, kind="stable")

    # snake-deal degree-sorted tiles to cores
    core_tiles = [[] for _ in range(NC)]
    for r in range(TPC):
        ts = list(range(r * NC, (r + 1) * NC))
        cs = list(range(NC)) if r % 2 == 0 else list(range(NC - 1, -1, -1))
        for c, t in zip(cs, ts):
            core_tiles[c].append(t)

    new_of_old = np.full(N, -1, np.int64)
    old_of_new = np.full(NPAD, -1, np.int64)
    for c in range(NC):
        for k, t in enumerate(core_tiles[c]):
            g0 = t * P
            members = order[g0: min(g0 + P, N)]
            base = c * SHARD + k * P
            idxs = base + np.arange(len(members))
            new_of_old[members] = idxs
            old_of_new[idxs] = members

    r_new = new_of_old[receivers]
    s_new = new_of_old[senders].astype(np.int32)

    # slot index of each edge within its receiver's list
    eorder = np.argsort(r_new, kind="stable")
    r_srt = r_new[eorder]
    run_start = np.searchsorted(r_srt, r_srt)
    slot_srt = np.arange(E) - run_start

    lane_cnt = np.bincount(r_new, minlength=NPAD)
    cnt = lane_cnt.reshape(NC, TPC, P)
    S = cnt.max(axis=(0, 2))          # per tile-position k, max over cores+lanes
    S = np.maximum(S, 1).astype(np.int64)
    bases = np.concatenate([[0], np.cumsum(S)])[:-1]
    TOT = int(S.sum())

    pad_rows = np.where(old_of_new < 0)[0]
    pad_idx = int(pad_rows[0]) if len(pad_rows) else 0

    ell = np.full((NC, P, TOT), pad_idx, np.int32)
    c_e = r_srt // SHARD
    k_e = (r_srt % SHARD) // P
    l_e = r_srt % P
    ell[c_e, l_e, bases[k_e] + slot_srt] = s_new[eorder]

    vmask = (old_of_new >= 0).astype(np.float32).reshape(NC, TPC, P)
    vmask = np.ascontiguousarray(np.transpose(vmask, (0, 2, 1)))  # [NC, P, TPC]

    G = int(node_graph_idx.max()) + 1 if node_graph_idx.size else 1
    onehotT = np.zeros((NC, P, TPC * G), np.float32)
    for c in range(NC):
        sl = slice(c * SHARD, (c + 1) * SHARD)
        olds = old_of_new[sl]
        real = olds >= 0
        kk = np.arange(SHARD) // P
        ll = np.arange(SHARD) % P
        gg = np.zeros(SHARD, np.int64)
        gg[real] = node_graph_idx[olds[real]]
        cols = kk * G + gg
        onehotT[c, ll[real], cols[real]] = 1.0

    return dict(
        TPC=TPC, NPAD=NPAD, SHARD=SHARD, S=S, bases=bases, TOT=TOT,
        new_of_old=new_of_old, old_of_new=old_of_new, ell=ell, vmask=vmask,
        onehotT=onehotT, pad_idx=pad_idx,
    )


# ---------------------------------------------------------------------------
# device program
def _build_nc(plan, D, H, G, GOUT, HOPS, a1_bias, coll_chunks=4, legalize=True,
              act_lrelu=True):
    TPC, SHARD, TOT = plan["TPC"], plan["SHARD"], plan["TOT"]
    S, bases = plan["S"], plan["bases"]
    NPAD = plan["NPAD"]
    QW = H + 1                    # q columns + a1 column
    YW = H + 1                    # y row width: q*v | v
    S_max = int(S.max())

    nc = bass.Bass()
    x0T_in = nc.dram_tensor("x0T", [D, SHARD], F32, kind="ExternalInput")
    wq_in = nc.dram_tensor("wq_eff", [HOPS, D, QW], F32, kind="ExternalInput")
    ell_in = nc.dram_tensor("ell", [P, TOT], I32, kind="ExternalInput")
    vmask_in = nc.dram_tensor("vmask", [P, TPC], F32, kind="ExternalInput")
    oh_in = nc.dram_tensor("onehotT", [P, TPC * G], F32, kind="ExternalInput")
    gT_in = nc.dram_tensor("globalsT", [D, G], F32, kind="ExternalInput")
    wg_in = nc.dram_tensor("Wg", [H + D, GOUT], F32, kind="ExternalInput")

    x_out = nc.dram_tensor("x_out", [SHARD, H], F32, kind="ExternalOutput")
    g_out = nc.dram_tensor("g_out", [G, GOUT], F32, kind="ExternalOutput")

    # chunk boundaries (in tiles) for the allgather overlap
    cb = [round(i * TPC / coll_chunks) for i in range(coll_chunks + 1)]
    y_shards = [
        nc.dram_tensor(f"y_shard_{i}", [(cb[i + 1] - cb[i]) * P, YW], F32)
        for i in range(coll_chunks)
    ]
    y_full = nc.dram_tensor("y_full", [NPAD, YW], F32, addr_space="Shared")
    agg_in = nc.dram_tensor("agg_in", [H, G], F32)
    agg_rd = nc.dram_tensor("agg_rd", [H, G], F32, addr_space="Shared")

    # physical row base of core c's chunk i rows inside y_full:
    # y_full layout = [chunk0: 8 cores x rows | chunk1: ... ]
    # ELL indices are precomputed against this layout on the host.

    with tile.TileContext(nc) as tc:
        with (
            tc.tile_pool(name="const", bufs=1) as cpool,
            tc.tile_pool(name="work", bufs=3) as wpool,
            tc.tile_pool(name="gather", bufs=12) as gpool,
            tc.tile_pool(name="psum", bufs=2, space="PSUM") as ppool,
            tc.tile_pool(name="psg", bufs=1, space="PSUM") as pgpool,
        ):
            ident = cpool.tile([P, P], F32)
            make_identity(nc, ident[:])

            wq_sb = []
            for h in range(HOPS):
                t = cpool.tile([D, QW], F32, tag=f"wq{h}")
                nc.sync.dma_start(out=t[:], in_=wq_in[h])
                wq_sb.append(t)
            ell_sb = cpool.tile([P, TOT], I32)
            nc.sync.dma_start(out=ell_sb[:], in_=ell_in[:])
            vmask_sb = cpool.tile([P, TPC], F32)
            nc.sync.dma_start(out=vmask_sb[:], in_=vmask_in[:])
            oh_sb = cpool.tile([P, TPC * G], F32)
            nc.sync.dma_start(out=oh_sb[:], in_=oh_in[:])
            gT_sb = cpool.tile([D, G], F32)
            nc.sync.dma_start(out=gT_sb[:], in_=gT_in[:])
            wg1_sb = cpool.tile([D, GOUT], F32, tag="wg1")
            nc.sync.dma_start(out=wg1_sb[:], in_=wg_in[:H, :])
            wg2_sb = cpool.tile([D, GOUT], F32, tag="wg2")
            nc.sync.dma_start(out=wg2_sb[:], in_=wg_in[H:, :])

            xT_a = cpool.tile([D, SHARD], F32, tag="xT_a")
            xT_b = cpool.tile([D, SHARD], F32, tag="xT_b")
            nc.sync.dma_start(out=xT_a[:], in_=x0T_in[:])

            # PE pre-touch of ident so later matmuls carry ident dep via
            # PE program order
            pre = ppool.tile([P, P], F32, tag="tr", space="PSUM")
            nc.tensor.transpose(out=pre[:], in_=ident[:], identity=ident[:])

            xT_cur, xT_next = xT_a, xT_b
            aggT_ps = None
            for h in range(HOPS):
                last = h == HOPS - 1
                # ---- Phase A: q, v, y rows for own shard ----
                for ci in range(coll_chunks):
                    for k in range(cb[ci], cb[ci + 1]):
                        qp = ppool.tile([P, QW], F32, tag="q", space="PSUM")
                        nc.tensor.matmul(
                            out=qp[:],
                            lhsT=xT_cur[:, k * P:(k + 1) * P],
                            rhs=wq_sb[h][:],
                            start=True, stop=True,
                        )
                        v = wpool.tile([P, 1], F32, tag="v")
                        nc.scalar.activation(
                            out=v[:], in_=qp[:, H:H + 1], func=AF.Exp,
                            bias=float(a1_bias[h]),
                        )
                        vm = wpool.tile([P, 1], F32, tag="vm")
                        nc.vector.tensor_tensor(
                            out=vm[:], in0=v[:], in1=vmask_sb[:, k:k + 1],
                            op=mybir.AluOpType.mult,
                        )
                        yt = wpool.tile([P, YW], F32, tag="yt")
                        nc.vector.tensor_scalar_mul(yt[:, :H], qp[:, :H], vm[:, :1])
                        nc.vector.tensor_copy(out=yt[:, H:H + 1], in_=vm[:])
                        kk = k - cb[ci]
                        nc.sync.dma_start(
                            out=y_shards[ci][kk * P:(kk + 1) * P, :], in_=yt[:],
                        )
                    # ---- Phase B (chunked): allgather this chunk ----
                    rows = (cb[ci + 1] - cb[ci]) * P
                    fbase = cb[ci] * P * NC
                    nc.gpsimd.collective_compute(
                        "AllGather",
                        mybir.AluOpType.bypass,
                        replica_groups=[list(range(NC))],
                        ins=[y_shards[ci][:]],
                        outs=[y_full[fbase:fbase + rows * NC, :]],
                    )

                # ---- Phase C: gather + aggregate per receiver tile ----
                for k in range(TPC):
                    Sk = int(S[k])
                    b0 = int(bases[k])
                    acc = ppool.tile([P, YW], F32, tag="acc", space="PSUM")
                    for j in range(Sk):
                        gb = gpool.tile([P, YW], F32, tag="gbuf")
                        nc.gpsimd.indirect_dma_start(
                            out=gb[:],
                            out_offset=None,
                            in_=y_full[:],
                            in_offset=bass.IndirectOffsetOnAxis(
                                ap=ell_sb[:, b0 + j:b0 + j + 1], axis=0,
                            ),
                        )
                        nc.tensor.matmul(
                            out=acc[:],
                            lhsT=ident[:],
                            rhs=gb[:],
                            start=(j == 0), stop=(j == Sk - 1),
                        )
                    den = wpool.tile([P, 1], F32, tag="den")
                    nc.vector.tensor_scalar_add(den[:], acc[:, H:H + 1], EPS)
                    rec = wpool.tile([P, 1], F32, tag="rec")
                    nc.vector.reciprocal(rec[:], den[:])
                    xn = wpool.tile([P, H], F32, tag="xn")
                    if act_lrelu:
                        nc.scalar.activation(
                            out=xn[:], in_=acc[:, :H], func=AF.Lrelu,
                            scale=rec[:, :1], alpha=LRELU_ALPHA,
                        )
                    else:
                        xsc = wpool.tile([P, H], F32, tag="xsc")
                        nc.vector.tensor_scalar_mul(xsc[:], acc[:, :H], rec[:, :1])
                        xsm = wpool.tile([P, H], F32, tag="xsm")
                        nc.vector.tensor_scalar_mul(xsm[:], xsc[:], LRELU_ALPHA)
                        nc.vector.tensor_tensor(
                            out=xn[:], in0=xsc[:], in1=xsm[:],
                            op=mybir.AluOpType.max,
                        )
                    if not last:
                        tr = ppool.tile([P, P], F32, tag="tr", space="PSUM")
                        nc.tensor.transpose(out=tr[:], in_=xn[:], identity=ident[:])
                        nc.vector.tensor_copy(
                            out=xT_next[:, k * P:(k + 1) * P], in_=tr[:],
                        )
                    else:
                        nc.sync.dma_start(
                            out=x_out[k * P:(k + 1) * P, :], in_=xn[:],
                        )
                        if aggT_ps is None:
                            aggT_ps = pgpool.tile([H, G], F32, tag="aggT",
                                                  space="PSUM")
                        nc.tensor.matmul(
                            out=aggT_ps[:],
                            lhsT=xn[:],
                            rhs=oh_sb[:, k * G:(k + 1) * G],
                            start=(k == 0), stop=(k == TPC - 1),
                        )
                xT_cur, xT_next = xT_next, xT_cur

            # ---- global update ----
            aggT_sb = wpool.tile([H, G], F32, tag="aggT_sb")
            nc.vector.tensor_copy(out=aggT_sb[:], in_=aggT_ps[:])
            nc.sync.dma_start(out=agg_in[:], in_=aggT_sb[:])
            nc.gpsimd.collective_compute(
                "AllReduce",
                mybir.AluOpType.add,
                replica_groups=[list(range(NC))],
                ins=[agg_in[:]],
                outs=[agg_rd[:]],
            )
            aggT2 = wpool.tile([H, G], F32, tag="aggT2")
            nc.sync.dma_start(out=aggT2[:], in_=agg_rd[:])
            gp = pgpool.tile([G, GOUT], F32, tag="gps", space="PSUM")
            nc.tensor.matmul(out=gp[:], lhsT=aggT2[:], rhs=wg1_sb[:],
                             start=True, stop=False)
            nc.tensor.matmul(out=gp[:], lhsT=gT_sb[:], rhs=wg2_sb[:],
                             start=False, stop=True)
            g_sb = wpool.tile([G, GOUT], F32, tag="g_sb")
            nc.vector.tensor_copy(out=g_sb[:], in_=gp[:])
            nc.sync.dma_start(out=g_out[:], in_=g_sb[:])

    if legalize:
        _legalize_waits(nc)
    return nc


# ---------------------------------------------------------------------------
def _remap_ell_for_chunks(plan, coll_chunks):
    """ELL indices address y_full whose layout is chunked:
    [chunk][core][rows_in_chunk]. Remap new-index -> physical row."""
    TPC, SHARD, NPAD = plan["TPC"], plan["SHARD"], plan["NPAD"]
    cb = [round(i * TPC / coll_chunks) for i in range(coll_chunks + 1)]
    newidx = np.arange(NPAD)
    c = newidx // SHARD
    r = newidx % SHARD
    k = r // P
    ci = np.searchsorted(np.asarray(cb), k, side="right") - 1
    cb_arr = np.asarray(cb)
    rows_per_core = (cb_arr[ci + 1] - cb_arr[ci]) * P
    fbase = cb_arr[ci] * P * NC
    within = r - cb_arr[ci] * P
    phys = fbase + c * rows_per_core + within
    return phys.astype(np.int32)


_CACHE = {}


def _get_compiled(key, builder):
    if key not in _CACHE:
        _CACHE[key] = builder()
    return _CACHE[key]


def _make_in_maps(inputs, plan, coll_chunks):
    nodes = np.asarray(inputs["nodes"], np.float32)
    Wq = np.asarray(inputs["Wq"])
    H = Wq.shape[2]
    D = nodes.shape[1]
    SHARD = plan["SHARD"]
    wa1 = np.asarray(inputs["Wa"])[:, :H, 0]
    wq_eff = np.concatenate(
        [Wq, np.einsum("hdk,hk->hd", Wq, wa1)[:, :, None]], axis=2
    ).astype(np.float32)
    phys = _remap_ell_for_chunks(plan, coll_chunks)
    ell_phys = phys[plan["ell"]]
    globalsT = np.ascontiguousarray(np.asarray(inputs["globals_"], np.float32).T)
    in_maps = []
    for c in range(NC):
        olds = plan["old_of_new"][c * SHARD:(c + 1) * SHARD]
        xs = np.zeros((SHARD, D), np.float32)
        real = olds >= 0
        xs[real] = nodes[olds[real]]
        in_maps.append({
            "x0T": np.ascontiguousarray(xs.T),
            "wq_eff": wq_eff,
            "ell": np.ascontiguousarray(ell_phys[c]),
            "vmask": plan["vmask"][c],
            "onehotT": plan["onehotT"][c],
            "globalsT": globalsT,
            "Wg": np.asarray(inputs["Wg"], np.float32),
        })
    return in_maps


def gat_kernel(nodes, globals_, Wq, bq, Wa, ba, Wg, bg, senders, receivers,
               node_graph_idx, tiles_per_core=49, coll_chunks=4, act_lrelu=True):
    N, D = nodes.shape
    HOPS, _, H = Wq.shape
    G, _ = globals_.shape
    GOUT = Wg.shape[1]

    senders = np.asarray(senders)
    receivers = np.asarray(receivers)
    node_graph_idx = np.asarray(node_graph_idx)
    nodes = np.asarray(nodes, np.float32)

    plan = _plan(senders, receivers, node_graph_idx, N, tiles_per_core)
    SHARD = plan["SHARD"]

    wa1 = np.asarray(Wa)[:, :H, 0]                       # [HOPS, H]
    a1_bias = tuple(float(b) for b in np.einsum("hk,hk->h", np.asarray(bq), wa1))
    assert not np.any(np.asarray(bq)), "bq != 0 unsupported in device program"
    # ba / Wa[:, H:] cancel inside the per-receiver softmax.

    nc = _get_compiled(
        ("nc", N, D, H, G, GOUT, HOPS, tiles_per_core, coll_chunks,
         plan["TOT"], tuple(plan["S"]), a1_bias, act_lrelu),
        lambda: _build_nc(plan, D, H, G, GOUT, HOPS, a1_bias, coll_chunks,
                          act_lrelu=act_lrelu),
    )

    in_maps = _make_in_maps(
        dict(nodes=nodes, globals_=globals_, Wq=Wq, Wa=Wa, Wg=Wg),
        plan, coll_chunks,
    )

    res = run_bass_kernel_spmd(nc, in_maps, list(range(NC))).results

    x_new = np.concatenate([res[c]["x_out"] for c in range(NC)], axis=0)
    x = x_new[plan["new_of_old"]]
    g = res[0]["g_out"]
    if np.any(np.asarray(bg)):
        g = g + np.asarray(bg)[None, :]
    return x.astype(np.float32), np.asarray(g, np.float32)


def kernel(**inputs):
    return gat_kernel(**inputs)
